# revision 32
# baseline (speedup 1.0000x reference)
"""HGT-style heterogeneous graph message passing on 8 Trainium2 cores.

Strategy (v2):
 - Host folds the per-(head, etype) relation transforms into per-(ntype, etype)
   64x128 weight matrices:  a_e = <k'_src, q_dst>  with
   k' = x @ Wk[nt] @ blockdiag_h(A A^T * pri / sqrt(d)),  m = x @ Wv[nt] @ blockdiag_h(M).
 - dst nodes are sharded across the 8 cores round-robin by degree rank, so all
   segment ops (softmax max/sum, weighted aggregation) become dense row
   reductions over degree-sorted [128, W_t] tiles.  No collectives.
 - Each core computes a deduplicated (src, etype) pair table [rows, 128] =
   [k' | m] on device (PE matmuls, bf16), then dma_gather's the rows of its
   slots in multi-tile slabs.
 - v2 vs v1: bf16 weights/features/table (4-8x faster PE, half the gather
   bytes), slab-batched gathers (fewer GPSIMD desc-gen calls), attention bias
   computed on device from degrees (no big abias input), un-replicated gather
   index upload, all inputs shipped bf16 where safe (~12 MB/core vs 30 MB).
"""

import sys

sys.path.insert(0, "/opt/trn_rl_repo")

import numpy as np
import ml_dtypes

BF16 = ml_dtypes.bfloat16

N, E = 40000, 640000
IN, H, HS = 64, 4, 16
NT, ET = 4, 8
D = H * HS  # 64
C = 8  # cores
NL = 5120  # padded local nodes per core
NTILES = NL // 128  # 40
NEG = -1.0e30
SLOT_BUDGET = 32  # max slab columns (tiles_in_slab * W4)

_cache = {}


def _host_prep(x, ntype, etype, src, dst):
    """Returns per-core input arrays + structural constants."""
    x = np.ascontiguousarray(np.asarray(x, dtype=np.float32))
    nt_ = np.asarray(ntype).astype(np.int64)
    et_ = np.asarray(etype).astype(np.int64)
    src = np.asarray(src).astype(np.int64)
    dst = np.asarray(dst).astype(np.int64)

    deg = np.bincount(dst, minlength=N)
    order = np.argsort(-deg, kind="stable")
    ranks = np.empty(N, dtype=np.int64)
    ranks[order] = np.arange(N)
    core_of_node = ranks % C
    local_of_node = ranks // C

    # tile widths (shared across cores): tile t covers global ranks [1024t, 1024(t+1))
    W = np.zeros(NTILES, dtype=np.int64)
    deg_by_rank = deg[order]
    for t in range(NTILES):
        lo, hi = t * 1024, min((t + 1) * 1024, N)
        W[t] = max(int(deg_by_rank[lo:hi].max()) if hi > lo else 1, 1)

    percore = []
    for c in range(C):
        ei = np.nonzero(core_of_node[dst] == c)[0]
        ld = local_of_node[dst[ei]]
        o = np.argsort(ld, kind="stable")
        percore.append((ei[o], ld[o]))

    # uniform 5-tile table chunks (8 chunks). Non-uniform variants (singleton
    # first chunks for a faster pipeline start) measured WORSE: small chunks
    # serialize their table-write -> gather dependency and double the
    # mid-stream GPSIMD idle gaps. Each chunk's padded pair count must stay
    # < 32000 (int16 gather idxs).
    tiles_of_chunk = [[0, 1], [2, 3, 4]] + [
        list(range(5 + h * 5, 10 + h * 5)) for h in range((NTILES - 5) // 5)
    ]
    NCH = len(tiles_of_chunk)
    CHOF = np.zeros(NTILES, dtype=np.int64)
    for h, ts_ in enumerate(tiles_of_chunk):
        CHOF[ts_] = h

    cnts = np.zeros((C, NCH, NT * ET), dtype=np.int64)
    pair_data = []
    for c in range(C):
        ei, ld = percore[c]
        ch_of = CHOF[ld // 128]
        key = src[ei] * ET + et_[ei]
        chunk_pairs = []
        for h in range(NCH):
            uk = np.unique(key[ch_of == h])  # sorted keys
            g = nt_[uk // ET] * ET + (uk % ET)
            np.add.at(cnts[c, h], g, 1)
            chunk_pairs.append((uk, g))
        pair_data.append(chunk_pairs)
    # 64-row group alignment: matmul output base partition must be 0/64
    R = 64 * ((cnts.max(axis=0) + 63) // 64)  # [NCH, 32]
    CHRs = 128 * ((R.sum(axis=1) + 127) // 128)
    if CHRs.max() >= 32000:
        raise RuntimeError("chunk too large for int16 gather indices")

    gbase = np.zeros((NCH, NT * ET), dtype=np.int64)
    for h in range(NCH):
        gbase[h] = np.concatenate(([0], np.cumsum(R[h])[:-1]))
    CHRs = [int(v) for v in CHRs]
    RB = np.concatenate(([0], np.cumsum(CHRs)[:-1])).astype(np.int64)
    RPtot = int(sum(CHRs))

    # slabs: group consecutive tiles of a chunk; slab columns = n_tiles * W4
    slabs = []  # (chunk, t0, n_tiles, W4, o8_offset)
    o8 = 0
    for h in range(NCH):
        ts_ = tiles_of_chunk[h]
        t, t_hi = ts_[0], ts_[-1] + 1
        while t < t_hi:
            W4 = int(W[t])
            n = 1
            while t + n < t_hi and (n + 1) * max(W4, int(W[t + n])) <= SLOT_BUDGET:
                W4 = max(W4, int(W[t + n]))
                n += 1
            slabs.append((h, t, n, W4, o8))
            o8 += 8 * n * W4
            t += n
    IDXWS = o8
    PBUD = max(s[2] * s[3] for s in slabs)
    SMAX = max(s[2] for s in slabs)
    WMAX = int(W.max())

    cores = []
    own_nodes = np.full((C, NL), -1, dtype=np.int64)
    for c in range(C):
        ei, ld = percore[c]
        etile = ld // 128
        ch_of = CHOF[etile]
        key = src[ei] * ET + et_[ei]

        ownc = order[c::C]
        own_nodes[c, : len(ownc)] = ownc

        rowid_of_edge = np.zeros(len(ei), dtype=np.int64)
        xp_node = np.full(RPtot, -1, dtype=np.int64)
        for h in range(NCH):
            uk, g = pair_data[c][h]  # uk sorted by key; g aligned
            po = np.argsort(g, kind="stable")
            gs = g[po]
            base_in_g = np.concatenate(
                ([0], np.cumsum(np.bincount(gs, minlength=NT * ET))[:-1])
            )
            rows_po = gbase[h][gs] + (np.arange(len(uk)) - base_in_g[gs])
            row_of_uk = np.empty(len(uk), dtype=np.int64)
            row_of_uk[po] = rows_po
            xp_node[RB[h] + row_of_uk] = uk // ET
            sel = np.nonzero(ch_of == h)[0]
            rowid_of_edge[sel] = row_of_uk[np.searchsorted(uk, key[sel])]

        xpT = np.zeros((IN, RPtot), dtype=np.float32)
        valid = xp_node >= 0
        xpT[:, valid] = x[xp_node[valid]].T
        xpT16 = xpT.astype(BF16)

        cnt = np.bincount(ld, minlength=NL)
        starts = np.concatenate(([0], np.cumsum(cnt)[:-1]))
        jpos = np.arange(len(ei)) - starts[ld]
        p_of = ld % 128

        # slab-batched gather indices: slot column c = i*W4 + j within slab,
        # list position k = c*128 + p, wrapped into 16 partitions.
        kmidx16 = np.zeros((16, IDXWS), dtype=np.int16)
        for h, t0, n, W4, o8s in slabs:
            idsl = np.zeros(n * W4 * 128, dtype=np.int16)
            sel = np.nonzero((etile >= t0) & (etile < t0 + n))[0]
            col = (etile[sel] - t0) * W4 + jpos[sel]
            idsl[col * 128 + p_of[sel]] = rowid_of_edge[sel].astype(np.int16)
            kmidx16[:, o8s : o8s + 8 * n * W4] = idsl.reshape(8 * n * W4, 16).T

        # attention bias per slab column: 0 for real slots, -1e30 for pads,
        # laid out to match the slab gather layout [128, IDXWS // 8]
        degl = np.zeros(NL, dtype=np.float32)
        nreal = len(ownc)
        degl[:nreal] = deg[ownc]
        abias = np.empty((128, IDXWS // 8), dtype=np.float32)
        for h, t0, n, W4, o8s in slabs:
            ob = o8s // 8
            dgs = degl.reshape(NTILES, 128)[t0 : t0 + n].T  # [128, n]
            cols = np.arange(W4)[None, None, :]  # j
            b = np.where(cols < dgs[:, :, None], 0.0, NEG).astype(np.float32)
            abias[:, ob : ob + n * W4] = b.reshape(128, n * W4)

        ntc = nt_[ownc]
        xo = x[ownc]
        xo4T = np.zeros((NT * IN, NL), dtype=np.float32)
        for t4 in range(NT):
            m4 = ntc == t4
            xo4T[t4 * IN : (t4 + 1) * IN, :nreal][:, m4] = xo[m4].T
        xo4T16 = xo4T.astype(BF16)

        oneh = np.zeros((NL, NT), dtype=np.float32)
        oneh[np.arange(nreal), ntc] = 1.0

        cores.append(dict(xpT16=xpT16, xo4T16=xo4T16, oneh=oneh, kmidx16=kmidx16, abias=abias))

    consts = dict(
        W=W, WMAX=WMAX, NCH=NCH, R=R, gbase=gbase, CHRs=CHRs,
        RB=RB, RPtot=RPtot, IDXWS=IDXWS, slabs=slabs, PBUD=PBUD, SMAX=SMAX,
        own_nodes=own_nodes, deg=deg,
    )
    return cores, consts


def _fold_weights(Wk, Wq, Wv, Wa, rel_att, rel_msg, rel_pri):
    Wk = np.asarray(Wk, np.float64)
    Wq = np.asarray(Wq, np.float64)
    Wv = np.asarray(Wv, np.float64)
    Wa = np.asarray(Wa, np.float64)
    rel_att = np.asarray(rel_att, np.float64)
    rel_msg = np.asarray(rel_msg, np.float64)
    rel_pri = np.asarray(rel_pri, np.float64)
    sd = float(np.sqrt(np.float32(HS)))

    wkm = np.zeros((IN, NT * ET, 2, D), np.float64)
    for nt in range(NT):
        for et in range(ET):
            Batt = np.zeros((D, D))
            Bmsg = np.zeros((D, D))
            for h in range(H):
                A = rel_att[h, et]
                Batt[h * HS : (h + 1) * HS, h * HS : (h + 1) * HS] = (
                    A @ A.T * rel_pri[h, et] / sd
                )
                Bmsg[h * HS : (h + 1) * HS, h * HS : (h + 1) * HS] = rel_msg[h, et]
            g = nt * ET + et
            wkm[:, g, 0] = Wk[nt] @ Batt
            wkm[:, g, 1] = Wv[nt] @ Bmsg
    wkm16 = wkm.reshape(IN, NT * ET * 2 * D).astype(BF16)
    wq416 = np.concatenate([Wq[t] for t in range(NT)], axis=0).astype(BF16)
    wa16 = np.concatenate([Wa[t] for t in range(NT)], axis=1).astype(BF16)
    return wkm16, wq416, wa16


def _build_program(consts):
    import concourse.mybir as mybir
    import concourse.tile as tile
    from concourse import bacc
    from concourse.masks import make_identity

    f32 = mybir.dt.float32
    bf16 = mybir.dt.bfloat16
    i16 = mybir.dt.int16
    W = consts["W"]
    WMAX = consts["WMAX"]
    NCH = consts["NCH"]
    R, gbase, CHRs, RB = consts["R"], consts["gbase"], consts["CHRs"], consts["RB"]
    RPtot, IDXWS = consts["RPtot"], consts["IDXWS"]
    slabs, PBUD, SMAX = consts["slabs"], consts["PBUD"], consts["SMAX"]

    nc = bacc.Bacc("TRN2", target_bir_lowering=False, debug=False, num_devices=C)

    xpT16 = nc.dram_tensor("xpT16", [IN, RPtot], bf16, kind="ExternalInput").ap()
    wkm16 = nc.dram_tensor("wkm16", [IN, NT * ET * 2 * D], bf16, kind="ExternalInput").ap()
    xo4T16 = nc.dram_tensor("xo4T16", [NT * IN, NL], bf16, kind="ExternalInput").ap()
    wq416 = nc.dram_tensor("wq416", [NT * IN, D], bf16, kind="ExternalInput").ap()
    wa16 = nc.dram_tensor("wa16", [D, NT * D], bf16, kind="ExternalInput").ap()
    oneh = nc.dram_tensor("oneh", [NL, NT], f32, kind="ExternalInput").ap()
    kmidx16 = nc.dram_tensor("kmidx16", [16, IDXWS], i16, kind="ExternalInput").ap()
    abias = nc.dram_tensor("abias", [128, IDXWS // 8], f32, kind="ExternalInput").ap()
    outp = nc.dram_tensor("outp", [NL, D], f32, kind="ExternalOutput").ap()
    kmtab = [
        nc.dram_tensor(f"kmtab{h}", [CHRs[h], 2 * D], bf16, kind="Internal").ap()
        for h in range(NCH)
    ]

    with tile.TileContext(nc) as tc:
        with tc.tile_pool(name="const", bufs=1) as constp, \
             tc.tile_pool(name="stage", bufs=2) as stage, \
             tc.tile_pool(name="work", bufs=3) as work, \
             tc.tile_pool(name="gwork", bufs=2) as gwork, \
             tc.tile_pool(name="gtp", bufs=8) as gtp, \
             tc.tile_pool(name="npsum", bufs=2, space="PSUM") as npsum, \
             tc.tile_pool(name="qpsum", bufs=2, space="PSUM") as qpsum, \
             tc.tile_pool(name="opsum", bufs=2, space="PSUM") as opsum:

            # ---- persistent constants ----
            wkm_s = constp.tile([IN, NT * ET * 2 * D], bf16, name="wkm_s", tag="wkm_s")
            nc.sync.dma_start(out=wkm_s[:], in_=wkm16[:, :])
            wq4_s = constp.tile([128, 2 * D], bf16, name="wq4_s", tag="wq4_s")
            for k in range(2):
                nc.sync.dma_start(
                    out=wq4_s[:, k * D : (k + 1) * D], in_=wq416[k * 128 : (k + 1) * 128, :]
                )
            wa_s = constp.tile([D, NT * D], bf16, name="wa_s", tag="wa_s")
            nc.sync.dma_start(out=wa_s[:], in_=wa16[:, :])
            oneh_s = constp.tile([128, NTILES * NT], f32, name="oneh_s", tag="oneh_s")
            nc.sync.dma_start(
                out=oneh_s[:].rearrange("p (t f) -> p t f", t=NTILES),
                in_=oneh[:, :].rearrange("(t p) f -> p t f", p=128),
            )
            abias_s = constp.tile([128, IDXWS // 8], f32, name="abias_s", tag="abias_s")
            nc.sync.dma_start(out=abias_s[:], in_=abias[:, :])
            kmidx_s = constp.tile([128, IDXWS], i16, name="kmidx_s", tag="kmidx_s")
            for rep in range(8):  # idxs wrapped in 16 partitions, replicated x8
                nc.sync.dma_start(
                    out=kmidx_s[16 * rep : 16 * (rep + 1)], in_=kmidx16[:, :]
                )
            ident = constp.tile([128, 128], bf16, name="ident", tag="ident")
            make_identity(nc, ident[:])
            qall = constp.tile([128, NTILES * D], bf16, name="qall", tag="qall")

            # ---- Q phase (4 node-tiles per load); emitted AFTER the first
            # table chunks so their DMAs/PE work aren't delayed ----
            def emit_q():
                for t0 in range(0, NTILES, 4):
                    x4_s = stage.tile([128, 2, 512], bf16, name=f"x4_{t0}", tag="x4")
                    nc.sync.dma_start(
                        out=x4_s[:],
                        in_=xo4T16[:, t0 * 128 : (t0 + 4) * 128].rearrange(
                            "(k p) n -> p k n", p=128
                        ),
                    )
                    for i in range(4):
                        t = t0 + i
                        q_p = qpsum.tile(
                            [128, D], f32, space="PSUM", name=f"q_p{t}", tag="q_p"
                        )
                        for k in range(2):
                            nc.tensor.matmul(
                                q_p[:],
                                lhsT=x4_s[:, k, i * 128 : (i + 1) * 128],
                                rhs=wq4_s[:, k * D : (k + 1) * D],
                                start=(k == 0),
                                stop=(k == 1),
                            )
                        nc.any.tensor_copy(
                            out=qall[:, t * D : (t + 1) * D], in_=q_p[:]
                        )

            # ---- node/pair-table phase per chunk ----
            def node_chunk_emitters(h):
                bounds = []  # (start_row, end_row, g) for nonempty groups
                for g in range(NT * ET):
                    if int(R[h, g]) > 0:
                        bounds.append((int(gbase[h, g]), int(gbase[h, g]) + int(R[h, g]), g))
                GT = bounds[-1][1]  # real rows (64-aligned)
                n_tiles = (GT + 127) // 128
                SLAB = 16
                emitters = []
                for s0 in range(0, n_tiles, SLAB):
                    emitters.append(
                        lambda s0=s0: node_slab(h, bounds, GT, n_tiles, SLAB, s0)
                    )
                return emitters

            def node_slab(h, bounds, GT, n_tiles, SLAB, s0):
                nb = min(SLAB, n_tiles - s0)
                row0 = s0 * 128
                rows = min(GT, (s0 + nb) * 128) - row0
                lhs_s = stage.tile(
                    [IN, SLAB * 128], bf16, name=f"lhs_{h}_{s0}", tag="lhs"
                )
                nc.sync.dma_start(
                    out=lhs_s[:, :rows],
                    in_=xpT16[:, int(RB[h]) + row0 : int(RB[h]) + row0 + rows],
                )
                slab = stage.tile(
                    [128, SLAB, 2 * D], bf16, name=f"slab_{h}_{s0}", tag="slab"
                )
                for i in range(0, nb, 4):
                    nn = min(4, nb - i)
                    km_p = npsum.tile(
                        [128, 512], f32, space="PSUM", name=f"km_p{h}_{s0}_{i}", tag="km_p"
                    )
                    covers = []
                    for j in range(nn):
                        t0 = row0 + (i + j) * 128  # tile's first table row
                        covers.append(min(128, GT - t0))
                        for gs, ge, g in bounds:
                            lo, hi = max(gs, t0), min(ge, t0 + 128)
                            if lo >= hi:
                                continue
                            nc.tensor.matmul(
                                km_p[lo - t0 : hi - t0, j * 128 : (j + 1) * 128],
                                lhsT=lhs_s[:, (i + j) * 128 + lo - t0 : (i + j) * 128 + hi - t0],
                                rhs=wkm_s[:, g * 128 : (g + 1) * 128],
                                start=True,
                                stop=True,
                            )
                    if covers[-1] == 128:
                        nc.any.tensor_copy(
                            out=slab[:, i : i + nn],
                            in_=km_p[:].rearrange("p (a d) -> p a d", a=4)[:, :nn],
                        )
                    else:
                        for j in range(nn):
                            nc.any.tensor_copy(
                                out=slab[: covers[j], i + j],
                                in_=km_p[: covers[j], j * 128 : (j + 1) * 128],
                            )
                nf = rows // 128  # full tiles in this slab
                if nf:
                    nc.sync.dma_start(
                        out=kmtab[h][row0 : row0 + nf * 128, :].rearrange(
                            "(a p) d -> p a d", p=128
                        ),
                        in_=slab[:, :nf],
                    )
                if rows % 128:
                    pr = rows % 128
                    nc.sync.dma_start(
                        out=kmtab[h][row0 + nf * 128 : row0 + rows, :],
                        in_=slab[:pr, nf],
                    )

            # ---- phase 3: slab of node-tiles: gather + softmax + aggregation ----
            def p3_slab(si):
                h, t0, n, W4, o8 = slabs[si]
                TOT = n * W4
                gt = gtp.tile([128, PBUD, 2 * D], bf16, name=f"gt{si}", tag="gt")
                nc.gpsimd.dma_gather(
                    out_ap=gt[:, :TOT],
                    in_ap=kmtab[h][:, :],
                    idxs_ap=kmidx_s[:, o8 : o8 + 8 * TOT],
                    num_idxs=128 * TOT,
                    num_idxs_reg=128 * TOT,
                    elem_size=2 * D,
                    single_packet=False,  # True crashes NRT for this shape
                )
                # aprod = k' * q  (bf16), whole slab in one op
                aprod = gwork.tile([128, PBUD, D], bf16, name=f"aprod{si}", tag="aprod")
                qb = (
                    qall[:, t0 * D : (t0 + n) * D]
                    .rearrange("p (s d) -> p s d", s=n)
                    .unsqueeze(2)
                    .to_broadcast([128, n, W4, D])
                )
                nc.vector.tensor_tensor(
                    out=aprod[:, :TOT].rearrange("p (s w) d -> p s w d", s=n),
                    in0=gt[:, :TOT, :D].rearrange("p (s w) d -> p s w d", s=n),
                    in1=qb,
                    op=mybir.AluOpType.mult,
                )
                # am[p, (s w), h] = sum_d aprod ; then += bias ; exp
                am = work.tile([128, PBUD, H], f32, name=f"am{si}", tag="am")
                for i in range(n):
                    nc.vector.tensor_reduce(
                        out=am[:, i * W4 : (i + 1) * W4],
                        in_=aprod[:, i * W4 : (i + 1) * W4].rearrange(
                            "p w (h d) -> p w h d", h=H
                        ),
                        axis=mybir.AxisListType.X,
                        op=mybir.AluOpType.add,
                    )
                ob = o8 // 8
                amb = work.tile([128, PBUD, H], f32, name=f"amb{si}", tag="amb")
                nc.vector.tensor_tensor(
                    out=amb[:, :TOT],
                    in0=am[:, :TOT],
                    in1=abias_s[:, ob : ob + TOT]
                    .unsqueeze(2)
                    .to_broadcast([128, TOT, H]),
                    op=mybir.AluOpType.add,
                )
                # softmax without max-subtraction: |a| is bounded well below
                # f32 exp overflow, and pads carry a -1e30 bias -> exp == 0.
                ex = work.tile([128, PBUD, H], bf16, name=f"ex{si}", tag="ex")
                nc.scalar.activation(
                    out=ex[:, :TOT], in_=amb[:, :TOT],
                    func=mybir.ActivationFunctionType.Exp,
                )
                den = work.tile([128, SMAX, H], f32, name=f"den{si}", tag="den")
                for i in range(n):
                    nc.vector.tensor_reduce(
                        out=den[:, i],
                        in_=ex[:, i * W4 : (i + 1) * W4].rearrange("p w h -> p h w"),
                        axis=mybir.AxisListType.X,
                        op=mybir.AluOpType.add,
                    )
                rden = work.tile([128, SMAX, H], f32, name=f"rden{si}", tag="rden")
                nc.vector.reciprocal(out=rden[:, :n], in_=den[:, :n])
                # mprod = m * exp(a) ; hm = sum_w mprod ; hm2 = hm / den
                mprod = gwork.tile([128, PBUD, D], bf16, name=f"mprod{si}", tag="mprod")
                hm = work.tile([128, SMAX, D], f32, name=f"hm{si}", tag="hm")
                for i in range(n):
                    nc.vector.tensor_tensor(
                        out=mprod[:, i * W4 : (i + 1) * W4].rearrange(
                            "p w (h d) -> p w h d", h=H
                        ),
                        in0=gt[:, i * W4 : (i + 1) * W4, D:].rearrange(
                            "p w (h d) -> p w h d", h=H
                        ),
                        in1=ex[:, i * W4 : (i + 1) * W4]
                        .unsqueeze(3)
                        .to_broadcast([128, W4, H, HS]),
                        op=mybir.AluOpType.mult,
                    )
                    nc.vector.tensor_reduce(
                        out=hm[:, i].rearrange("p (h d) -> p h d", h=H),
                        in_=mprod[:, i * W4 : (i + 1) * W4].rearrange(
                            "p w (h d) -> p h d w", h=H
                        ),
                        axis=mybir.AxisListType.X,
                        op=mybir.AluOpType.add,
                    )
                hm2 = work.tile([128, SMAX, D], bf16, name=f"hm2{si}", tag="hm2")
                nc.vector.tensor_tensor(
                    out=hm2[:, :n].rearrange("p s (h d) -> p s h d", h=H),
                    in0=hm[:, :n].rearrange("p s (h d) -> p s h d", h=H),
                    in1=rden[:, :n].unsqueeze(3).to_broadcast([128, n, H, HS]),
                    op=mybir.AluOpType.mult,
                )
                # output projection per tile
                for i in range(n):
                    t = t0 + i
                    tp = opsum.tile([128, 128], bf16, space="PSUM", name=f"tp{t}", tag="tp")
                    nc.tensor.transpose(out=tp[:D, :], in_=hm2[:, i], identity=ident[:])
                    hT = work.tile([D, 128], bf16, name=f"hT{t}", tag="hT")
                    nc.any.tensor_copy(out=hT[:], in_=tp[:D, :])
                    o4 = opsum.tile([128, NT * D], f32, space="PSUM", name=f"o4_{t}", tag="o4")
                    nc.tensor.matmul(o4[:], lhsT=hT[:], rhs=wa_s[:], start=True, stop=True)
                    osel = work.tile([128, NT * D], f32, name=f"osel{t}", tag="osel")
                    ohb = (
                        oneh_s[:]
                        .rearrange("p (t f) -> p t f", t=NTILES)[:, t]
                        .unsqueeze(1)
                        .to_broadcast([128, D, NT])
                    )
                    nc.vector.tensor_tensor(
                        out=osel[:].rearrange("p (t d) -> p d t", t=NT),
                        in0=o4[:].rearrange("p (t d) -> p d t", t=NT),
                        in1=ohb,
                        op=mybir.AluOpType.mult,
                    )
                    ot = work.tile([128, D], f32, name=f"ot{t}", tag="ot")
                    nc.vector.tensor_reduce(
                        out=ot[:],
                        in_=osel[:].rearrange("p (t d) -> p d t", t=NT),
                        axis=mybir.AxisListType.X,
                        op=mybir.AluOpType.add,
                    )
                    nc.sync.dma_start(out=outp[t * 128 : (t + 1) * 128, :], in_=ot[:])

            # emission order = scheduler priority: build chunk 0's table first,
            # then interleave later chunks' table slabs with phase 3 of the
            # already-built chunks so DMA/PE/DVE overlap across phases.
            # table build runs TWO chunks ahead of phase 3 so a chunk's last
            # kmtab write lands well before its first gather needs it
            slabs_of_chunk = [
                [si for si, s in enumerate(slabs) if s[0] == h] for h in range(NCH)
            ]
            for em in node_chunk_emitters(0):
                em()
            for em in node_chunk_emitters(1):
                em()
            emit_q()
            for h in range(2, NCH):
                ems = node_chunk_emitters(h)
                tiles = slabs_of_chunk[h - 2]
                ns, ntl = len(ems), len(tiles)
                si_ = ti = 0
                while si_ < ns or ti < ntl:
                    take = (si_ + 1) * ntl <= (ti + 1) * ns
                    if si_ < ns and (take or ti >= ntl):
                        ems[si_]()
                        si_ += 1
                    else:
                        p3_slab(tiles[ti])
                        ti += 1
            for si in slabs_of_chunk[NCH - 2] + slabs_of_chunk[NCH - 1]:
                p3_slab(si)

    nc.compile()
    return nc


def kernel(x, ntype, etype, src, dst, Wk, Wq, Wv, Wa, rel_att, rel_msg, rel_pri):
    import os

    from concourse import bass_utils

    cores, consts = _host_prep(x, ntype, etype, src, dst)
    wkm16, wq416, wa16 = _fold_weights(Wk, Wq, Wv, Wa, rel_att, rel_msg, rel_pri)

    struct_sig = (
        tuple(consts["W"].tolist()),
        consts["NCH"],
        tuple(consts["CHRs"]),
        tuple(consts["R"].ravel().tolist()),
        tuple(consts["slabs"]),
    )
    if "prog" not in _cache or _cache["prog"][0] != struct_sig:
        _cache["prog"] = (struct_sig, _build_program(consts))
    nc = _cache["prog"][1]

    in_maps = [
        dict(
            xpT16=d["xpT16"], wkm16=wkm16, xo4T16=d["xo4T16"], wq416=wq416,
            wa16=wa16, oneh=d["oneh"], kmidx16=d["kmidx16"], abias=d["abias"],
        )
        for d in cores
    ]
    trace_kw = {}
    if os.environ.get("GNN_TRACE") == "1":
        trace_kw = dict(trace=True, tmpdir=os.environ.get("GNN_TRACE_DIR") or None)
    res = bass_utils.run_bass_kernel_spmd(
        nc, in_maps, core_ids=list(range(C)), **trace_kw
    )
    _cache["last_res"] = res

    out = np.zeros((N, D), dtype=np.float32)
    own = consts["own_nodes"]
    for c in range(C):
        oc = res.results[c]["outp"]
        m = own[c] >= 0
        out[own[c][m]] = oc[m]
    out[consts["deg"] == 0] = 0.0
    return out


# revision 34
# speedup vs baseline: 1.0507x; 1.0507x over previous
"""HGT-style heterogeneous graph message passing on 8 Trainium2 cores.

Strategy (v2):
 - Host folds the per-(head, etype) relation transforms into per-(ntype, etype)
   64x128 weight matrices:  a_e = <k'_src, q_dst>  with
   k' = x @ Wk[nt] @ blockdiag_h(A A^T * pri / sqrt(d)),  m = x @ Wv[nt] @ blockdiag_h(M).
 - dst nodes are sharded across the 8 cores round-robin by degree rank, so all
   segment ops (softmax max/sum, weighted aggregation) become dense row
   reductions over degree-sorted [128, W_t] tiles.  No collectives.
 - Each core computes a deduplicated (src, etype) pair table [rows, 128] =
   [k' | m] on device (PE matmuls, bf16), then dma_gather's the rows of its
   slots in multi-tile slabs.
 - v2 vs v1: bf16 weights/features/table (4-8x faster PE, half the gather
   bytes), slab-batched gathers (fewer GPSIMD desc-gen calls), attention bias
   computed on device from degrees (no big abias input), un-replicated gather
   index upload, all inputs shipped bf16 where safe (~12 MB/core vs 30 MB).
"""

import sys

sys.path.insert(0, "/opt/trn_rl_repo")

import numpy as np
import ml_dtypes

BF16 = ml_dtypes.bfloat16

N, E = 40000, 640000
IN, H, HS = 64, 4, 16
NT, ET = 4, 8
D = H * HS  # 64
C = 8  # cores
NL = 5120  # padded local nodes per core
NTILES = NL // 128  # 40
NEG = -1.0e30
SLOT_BUDGET = 32  # max slab columns (tiles_in_slab * W4)

_cache = {}


def _host_prep(x, ntype, etype, src, dst):
    """Returns per-core input arrays + structural constants."""
    x = np.ascontiguousarray(np.asarray(x, dtype=np.float32))
    nt_ = np.asarray(ntype).astype(np.int64)
    et_ = np.asarray(etype).astype(np.int64)
    src = np.asarray(src).astype(np.int64)
    dst = np.asarray(dst).astype(np.int64)

    deg = np.bincount(dst, minlength=N)
    order = np.argsort(-deg, kind="stable")
    ranks = np.empty(N, dtype=np.int64)
    ranks[order] = np.arange(N)
    core_of_node = ranks % C
    local_of_node = ranks // C

    # tile widths (shared across cores): tile t covers global ranks [1024t, 1024(t+1))
    W = np.zeros(NTILES, dtype=np.int64)
    deg_by_rank = deg[order]
    for t in range(NTILES):
        lo, hi = t * 1024, min((t + 1) * 1024, N)
        W[t] = max(int(deg_by_rank[lo:hi].max()) if hi > lo else 1, 1)

    percore = []
    for c in range(C):
        ei = np.nonzero(core_of_node[dst] == c)[0]
        ld = local_of_node[dst[ei]]
        o = np.argsort(ld, kind="stable")
        percore.append((ei[o], ld[o]))

    # uniform 5-tile table chunks (8 chunks). Non-uniform variants (singleton
    # first chunks for a faster pipeline start) measured WORSE: small chunks
    # serialize their table-write -> gather dependency and double the
    # mid-stream GPSIMD idle gaps. Each chunk's padded pair count must stay
    # < 32000 (int16 gather idxs).
    tiles_of_chunk = [list(range(h * 5, (h + 1) * 5)) for h in range(NTILES // 5)]
    NCH = len(tiles_of_chunk)
    CHOF = np.zeros(NTILES, dtype=np.int64)
    for h, ts_ in enumerate(tiles_of_chunk):
        CHOF[ts_] = h

    cnts = np.zeros((C, NCH, NT * ET), dtype=np.int64)
    pair_data = []
    for c in range(C):
        ei, ld = percore[c]
        ch_of = CHOF[ld // 128]
        key = src[ei] * ET + et_[ei]
        chunk_pairs = []
        for h in range(NCH):
            uk = np.unique(key[ch_of == h])  # sorted keys
            g = nt_[uk // ET] * ET + (uk % ET)
            np.add.at(cnts[c, h], g, 1)
            chunk_pairs.append((uk, g))
        pair_data.append(chunk_pairs)
    # 64-row group alignment: matmul output base partition must be 0/64
    R = 64 * ((cnts.max(axis=0) + 63) // 64)  # [NCH, 32]
    CHRs = 128 * ((R.sum(axis=1) + 127) // 128)
    if CHRs.max() >= 32000:
        raise RuntimeError("chunk too large for int16 gather indices")

    gbase = np.zeros((NCH, NT * ET), dtype=np.int64)
    for h in range(NCH):
        gbase[h] = np.concatenate(([0], np.cumsum(R[h])[:-1]))
    CHRs = [int(v) for v in CHRs]
    RB = np.concatenate(([0], np.cumsum(CHRs)[:-1])).astype(np.int64)
    RPtot = int(sum(CHRs))

    # slabs: group consecutive tiles of a chunk; slab columns = n_tiles * W4
    slabs = []  # (chunk, t0, n_tiles, W4, o8_offset)
    o8 = 0
    for h in range(NCH):
        ts_ = tiles_of_chunk[h]
        t, t_hi = ts_[0], ts_[-1] + 1
        while t < t_hi:
            W4 = int(W[t])
            n = 1
            while t + n < t_hi and (n + 1) * max(W4, int(W[t + n])) <= SLOT_BUDGET:
                W4 = max(W4, int(W[t + n]))
                n += 1
            slabs.append((h, t, n, W4, o8))
            o8 += 8 * n * W4
            t += n
    IDXWS = o8
    PBUD = max(s[2] * s[3] for s in slabs)
    SMAX = max(s[2] for s in slabs)
    WMAX = int(W.max())

    cores = []
    own_nodes = np.full((C, NL), -1, dtype=np.int64)
    for c in range(C):
        ei, ld = percore[c]
        etile = ld // 128
        ch_of = CHOF[etile]
        key = src[ei] * ET + et_[ei]

        ownc = order[c::C]
        own_nodes[c, : len(ownc)] = ownc

        rowid_of_edge = np.zeros(len(ei), dtype=np.int64)
        xp_node = np.full(RPtot, -1, dtype=np.int64)
        for h in range(NCH):
            uk, g = pair_data[c][h]  # uk sorted by key; g aligned
            po = np.argsort(g, kind="stable")
            gs = g[po]
            base_in_g = np.concatenate(
                ([0], np.cumsum(np.bincount(gs, minlength=NT * ET))[:-1])
            )
            rows_po = gbase[h][gs] + (np.arange(len(uk)) - base_in_g[gs])
            row_of_uk = np.empty(len(uk), dtype=np.int64)
            row_of_uk[po] = rows_po
            xp_node[RB[h] + row_of_uk] = uk // ET
            sel = np.nonzero(ch_of == h)[0]
            rowid_of_edge[sel] = row_of_uk[np.searchsorted(uk, key[sel])]

        xpT = np.zeros((IN, RPtot), dtype=np.float32)
        valid = xp_node >= 0
        xpT[:, valid] = x[xp_node[valid]].T
        xpT16 = xpT.astype(BF16)

        cnt = np.bincount(ld, minlength=NL)
        starts = np.concatenate(([0], np.cumsum(cnt)[:-1]))
        jpos = np.arange(len(ei)) - starts[ld]
        p_of = ld % 128

        # slab-batched gather indices: slot column c = i*W4 + j within slab,
        # list position k = c*128 + p, wrapped into 16 partitions.
        kmidx16 = np.zeros((16, IDXWS), dtype=np.int16)
        for h, t0, n, W4, o8s in slabs:
            idsl = np.zeros(n * W4 * 128, dtype=np.int16)
            sel = np.nonzero((etile >= t0) & (etile < t0 + n))[0]
            col = (etile[sel] - t0) * W4 + jpos[sel]
            idsl[col * 128 + p_of[sel]] = rowid_of_edge[sel].astype(np.int16)
            kmidx16[:, o8s : o8s + 8 * n * W4] = idsl.reshape(8 * n * W4, 16).T

        # attention bias per slab column: 0 for real slots, -1e30 for pads,
        # laid out to match the slab gather layout [128, IDXWS // 8]
        degl = np.zeros(NL, dtype=np.float32)
        nreal = len(ownc)
        degl[:nreal] = deg[ownc]
        abias = np.empty((128, IDXWS // 8), dtype=np.float32)
        for h, t0, n, W4, o8s in slabs:
            ob = o8s // 8
            dgs = degl.reshape(NTILES, 128)[t0 : t0 + n].T  # [128, n]
            cols = np.arange(W4)[None, None, :]  # j
            b = np.where(cols < dgs[:, :, None], 0.0, NEG).astype(np.float32)
            abias[:, ob : ob + n * W4] = b.reshape(128, n * W4)

        ntc = nt_[ownc]
        xo = x[ownc]
        xo4T = np.zeros((NT * IN, NL), dtype=np.float32)
        for t4 in range(NT):
            m4 = ntc == t4
            xo4T[t4 * IN : (t4 + 1) * IN, :nreal][:, m4] = xo[m4].T
        xo4T16 = xo4T.astype(BF16)

        oneh = np.zeros((NL, NT), dtype=np.float32)
        oneh[np.arange(nreal), ntc] = 1.0

        cores.append(dict(xpT16=xpT16, xo4T16=xo4T16, oneh=oneh, kmidx16=kmidx16, abias=abias))

    consts = dict(
        W=W, WMAX=WMAX, NCH=NCH, R=R, gbase=gbase, CHRs=CHRs,
        RB=RB, RPtot=RPtot, IDXWS=IDXWS, slabs=slabs, PBUD=PBUD, SMAX=SMAX,
        own_nodes=own_nodes, deg=deg,
    )
    return cores, consts


def _fold_weights(Wk, Wq, Wv, Wa, rel_att, rel_msg, rel_pri):
    Wk = np.asarray(Wk, np.float64)
    Wq = np.asarray(Wq, np.float64)
    Wv = np.asarray(Wv, np.float64)
    Wa = np.asarray(Wa, np.float64)
    rel_att = np.asarray(rel_att, np.float64)
    rel_msg = np.asarray(rel_msg, np.float64)
    rel_pri = np.asarray(rel_pri, np.float64)
    sd = float(np.sqrt(np.float32(HS)))

    wkm = np.zeros((IN, NT * ET, 2, D), np.float64)
    for nt in range(NT):
        for et in range(ET):
            Batt = np.zeros((D, D))
            Bmsg = np.zeros((D, D))
            for h in range(H):
                A = rel_att[h, et]
                Batt[h * HS : (h + 1) * HS, h * HS : (h + 1) * HS] = (
                    A @ A.T * rel_pri[h, et] / sd
                )
                Bmsg[h * HS : (h + 1) * HS, h * HS : (h + 1) * HS] = rel_msg[h, et]
            g = nt * ET + et
            wkm[:, g, 0] = Wk[nt] @ Batt
            wkm[:, g, 1] = Wv[nt] @ Bmsg
    wkm16 = wkm.reshape(IN, NT * ET * 2 * D).astype(BF16)
    wq416 = np.concatenate([Wq[t] for t in range(NT)], axis=0).astype(BF16)
    wa16 = np.concatenate([Wa[t] for t in range(NT)], axis=1).astype(BF16)
    return wkm16, wq416, wa16


def _build_program(consts):
    import concourse.mybir as mybir
    import concourse.tile as tile
    from concourse import bacc
    from concourse.masks import make_identity

    f32 = mybir.dt.float32
    bf16 = mybir.dt.bfloat16
    i16 = mybir.dt.int16
    W = consts["W"]
    WMAX = consts["WMAX"]
    NCH = consts["NCH"]
    R, gbase, CHRs, RB = consts["R"], consts["gbase"], consts["CHRs"], consts["RB"]
    RPtot, IDXWS = consts["RPtot"], consts["IDXWS"]
    slabs, PBUD, SMAX = consts["slabs"], consts["PBUD"], consts["SMAX"]

    nc = bacc.Bacc("TRN2", target_bir_lowering=False, debug=False, num_devices=C)

    xpT16 = nc.dram_tensor("xpT16", [IN, RPtot], bf16, kind="ExternalInput").ap()
    wkm16 = nc.dram_tensor("wkm16", [IN, NT * ET * 2 * D], bf16, kind="ExternalInput").ap()
    xo4T16 = nc.dram_tensor("xo4T16", [NT * IN, NL], bf16, kind="ExternalInput").ap()
    wq416 = nc.dram_tensor("wq416", [NT * IN, D], bf16, kind="ExternalInput").ap()
    wa16 = nc.dram_tensor("wa16", [D, NT * D], bf16, kind="ExternalInput").ap()
    oneh = nc.dram_tensor("oneh", [NL, NT], f32, kind="ExternalInput").ap()
    kmidx16 = nc.dram_tensor("kmidx16", [16, IDXWS], i16, kind="ExternalInput").ap()
    abias = nc.dram_tensor("abias", [128, IDXWS // 8], f32, kind="ExternalInput").ap()
    outp = nc.dram_tensor("outp", [NL, D], f32, kind="ExternalOutput").ap()
    kmtab = [
        nc.dram_tensor(f"kmtab{h}", [CHRs[h], 2 * D], bf16, kind="Internal").ap()
        for h in range(NCH)
    ]

    with tile.TileContext(nc) as tc:
        with tc.tile_pool(name="const", bufs=1) as constp, \
             tc.tile_pool(name="stage", bufs=3) as stage, \
             tc.tile_pool(name="work", bufs=3) as work, \
             tc.tile_pool(name="gwork", bufs=2) as gwork, \
             tc.tile_pool(name="gtp", bufs=8) as gtp, \
             tc.tile_pool(name="npsum", bufs=2, space="PSUM") as npsum, \
             tc.tile_pool(name="qpsum", bufs=2, space="PSUM") as qpsum, \
             tc.tile_pool(name="opsum", bufs=2, space="PSUM") as opsum:

            # ---- persistent constants ----
            wkm_s = constp.tile([IN, NT * ET * 2 * D], bf16, name="wkm_s", tag="wkm_s")
            nc.sync.dma_start(out=wkm_s[:], in_=wkm16[:, :])
            wq4_s = constp.tile([128, 2 * D], bf16, name="wq4_s", tag="wq4_s")
            for k in range(2):
                nc.sync.dma_start(
                    out=wq4_s[:, k * D : (k + 1) * D], in_=wq416[k * 128 : (k + 1) * 128, :]
                )
            wa_s = constp.tile([D, NT * D], bf16, name="wa_s", tag="wa_s")
            nc.sync.dma_start(out=wa_s[:], in_=wa16[:, :])
            oneh_s = constp.tile([128, NTILES * NT], f32, name="oneh_s", tag="oneh_s")
            nc.sync.dma_start(
                out=oneh_s[:].rearrange("p (t f) -> p t f", t=NTILES),
                in_=oneh[:, :].rearrange("(t p) f -> p t f", p=128),
            )
            abias_s = constp.tile([128, IDXWS // 8], f32, name="abias_s", tag="abias_s")
            nc.sync.dma_start(out=abias_s[:], in_=abias[:, :])
            kmidx_s = constp.tile([128, IDXWS], i16, name="kmidx_s", tag="kmidx_s")
            for rep in range(8):  # idxs wrapped in 16 partitions, replicated x8
                nc.sync.dma_start(
                    out=kmidx_s[16 * rep : 16 * (rep + 1)], in_=kmidx16[:, :]
                )
            ident = constp.tile([128, 128], bf16, name="ident", tag="ident")
            make_identity(nc, ident[:])
            qall = constp.tile([128, NTILES * D], bf16, name="qall", tag="qall")

            # ---- Q phase (4 node-tiles per load); emitted AFTER the first
            # table chunks so their DMAs/PE work aren't delayed ----
            def emit_q():
                for t0 in range(0, NTILES, 4):
                    x4_s = stage.tile([128, 2, 512], bf16, name=f"x4_{t0}", tag="x4")
                    nc.sync.dma_start(
                        out=x4_s[:],
                        in_=xo4T16[:, t0 * 128 : (t0 + 4) * 128].rearrange(
                            "(k p) n -> p k n", p=128
                        ),
                    )
                    for i in range(4):
                        t = t0 + i
                        q_p = qpsum.tile(
                            [128, D], f32, space="PSUM", name=f"q_p{t}", tag="q_p"
                        )
                        for k in range(2):
                            nc.tensor.matmul(
                                q_p[:],
                                lhsT=x4_s[:, k, i * 128 : (i + 1) * 128],
                                rhs=wq4_s[:, k * D : (k + 1) * D],
                                start=(k == 0),
                                stop=(k == 1),
                            )
                        nc.any.tensor_copy(
                            out=qall[:, t * D : (t + 1) * D], in_=q_p[:]
                        )

            # ---- node/pair-table phase per chunk ----
            def node_chunk_emitters(h):
                bounds = []  # (start_row, end_row, g) for nonempty groups
                for g in range(NT * ET):
                    if int(R[h, g]) > 0:
                        bounds.append((int(gbase[h, g]), int(gbase[h, g]) + int(R[h, g]), g))
                GT = bounds[-1][1]  # real rows (64-aligned)
                n_tiles = (GT + 127) // 128
                SLAB = 16
                emitters = []
                for s0 in range(0, n_tiles, SLAB):
                    emitters.append(
                        lambda s0=s0: node_slab(h, bounds, GT, n_tiles, SLAB, s0)
                    )
                return emitters

            def node_slab(h, bounds, GT, n_tiles, SLAB, s0):
                nb = min(SLAB, n_tiles - s0)
                row0 = s0 * 128
                rows = min(GT, (s0 + nb) * 128) - row0
                lhs_s = stage.tile(
                    [IN, SLAB * 128], bf16, name=f"lhs_{h}_{s0}", tag="lhs"
                )
                nc.sync.dma_start(
                    out=lhs_s[:, :rows],
                    in_=xpT16[:, int(RB[h]) + row0 : int(RB[h]) + row0 + rows],
                )
                slab = stage.tile(
                    [128, SLAB, 2 * D], bf16, name=f"slab_{h}_{s0}", tag="slab"
                )
                for i in range(0, nb, 4):
                    nn = min(4, nb - i)
                    km_p = npsum.tile(
                        [128, 512], f32, space="PSUM", name=f"km_p{h}_{s0}_{i}", tag="km_p"
                    )
                    covers = []
                    for j in range(nn):
                        t0 = row0 + (i + j) * 128  # tile's first table row
                        covers.append(min(128, GT - t0))
                        for gs, ge, g in bounds:
                            lo, hi = max(gs, t0), min(ge, t0 + 128)
                            if lo >= hi:
                                continue
                            nc.tensor.matmul(
                                km_p[lo - t0 : hi - t0, j * 128 : (j + 1) * 128],
                                lhsT=lhs_s[:, (i + j) * 128 + lo - t0 : (i + j) * 128 + hi - t0],
                                rhs=wkm_s[:, g * 128 : (g + 1) * 128],
                                start=True,
                                stop=True,
                            )
                    if covers[-1] == 128:
                        nc.any.tensor_copy(
                            out=slab[:, i : i + nn],
                            in_=km_p[:].rearrange("p (a d) -> p a d", a=4)[:, :nn],
                        )
                    else:
                        for j in range(nn):
                            nc.any.tensor_copy(
                                out=slab[: covers[j], i + j],
                                in_=km_p[: covers[j], j * 128 : (j + 1) * 128],
                            )
                nf = rows // 128  # full tiles in this slab
                if nf:
                    nc.sync.dma_start(
                        out=kmtab[h][row0 : row0 + nf * 128, :].rearrange(
                            "(a p) d -> p a d", p=128
                        ),
                        in_=slab[:, :nf],
                    )
                if rows % 128:
                    pr = rows % 128
                    nc.sync.dma_start(
                        out=kmtab[h][row0 + nf * 128 : row0 + rows, :],
                        in_=slab[:pr, nf],
                    )

            # ---- phase 3: slab of node-tiles: gather + softmax + aggregation ----
            def p3_slab(si):
                h, t0, n, W4, o8 = slabs[si]
                TOT = n * W4
                gt = gtp.tile([128, PBUD, 2 * D], bf16, name=f"gt{si}", tag="gt")
                nc.gpsimd.dma_gather(
                    out_ap=gt[:, :TOT],
                    in_ap=kmtab[h][:, :],
                    idxs_ap=kmidx_s[:, o8 : o8 + 8 * TOT],
                    num_idxs=128 * TOT,
                    num_idxs_reg=128 * TOT,
                    elem_size=2 * D,
                    single_packet=False,  # True crashes NRT for this shape
                )
                # aprod = k' * q  (bf16), whole slab in one op
                aprod = gwork.tile([128, PBUD, D], bf16, name=f"aprod{si}", tag="aprod")
                qb = (
                    qall[:, t0 * D : (t0 + n) * D]
                    .rearrange("p (s d) -> p s d", s=n)
                    .unsqueeze(2)
                    .to_broadcast([128, n, W4, D])
                )
                nc.vector.tensor_tensor(
                    out=aprod[:, :TOT].rearrange("p (s w) d -> p s w d", s=n),
                    in0=gt[:, :TOT, :D].rearrange("p (s w) d -> p s w d", s=n),
                    in1=qb,
                    op=mybir.AluOpType.mult,
                )
                # am[p, (s w), h] = sum_d aprod ; then += bias ; exp
                am = work.tile([128, PBUD, H], f32, name=f"am{si}", tag="am")
                for i in range(n):
                    nc.vector.tensor_reduce(
                        out=am[:, i * W4 : (i + 1) * W4],
                        in_=aprod[:, i * W4 : (i + 1) * W4].rearrange(
                            "p w (h d) -> p w h d", h=H
                        ),
                        axis=mybir.AxisListType.X,
                        op=mybir.AluOpType.add,
                    )
                ob = o8 // 8
                amb = work.tile([128, PBUD, H], f32, name=f"amb{si}", tag="amb")
                nc.vector.tensor_tensor(
                    out=amb[:, :TOT],
                    in0=am[:, :TOT],
                    in1=abias_s[:, ob : ob + TOT]
                    .unsqueeze(2)
                    .to_broadcast([128, TOT, H]),
                    op=mybir.AluOpType.add,
                )
                # softmax without max-subtraction: |a| is bounded well below
                # f32 exp overflow, and pads carry a -1e30 bias -> exp == 0.
                ex = work.tile([128, PBUD, H], bf16, name=f"ex{si}", tag="ex")
                nc.scalar.activation(
                    out=ex[:, :TOT], in_=amb[:, :TOT],
                    func=mybir.ActivationFunctionType.Exp,
                )
                den = work.tile([128, SMAX, H], f32, name=f"den{si}", tag="den")
                for i in range(n):
                    nc.vector.tensor_reduce(
                        out=den[:, i],
                        in_=ex[:, i * W4 : (i + 1) * W4].rearrange("p w h -> p h w"),
                        axis=mybir.AxisListType.X,
                        op=mybir.AluOpType.add,
                    )
                rden = work.tile([128, SMAX, H], f32, name=f"rden{si}", tag="rden")
                nc.vector.reciprocal(out=rden[:, :n], in_=den[:, :n])
                # mprod = m * exp(a) ; hm = sum_w mprod ; hm2 = hm / den
                mprod = gwork.tile([128, PBUD, D], bf16, name=f"mprod{si}", tag="mprod")
                hm = work.tile([128, SMAX, D], f32, name=f"hm{si}", tag="hm")
                for i in range(n):
                    nc.vector.tensor_tensor(
                        out=mprod[:, i * W4 : (i + 1) * W4].rearrange(
                            "p w (h d) -> p w h d", h=H
                        ),
                        in0=gt[:, i * W4 : (i + 1) * W4, D:].rearrange(
                            "p w (h d) -> p w h d", h=H
                        ),
                        in1=ex[:, i * W4 : (i + 1) * W4]
                        .unsqueeze(3)
                        .to_broadcast([128, W4, H, HS]),
                        op=mybir.AluOpType.mult,
                    )
                    nc.vector.tensor_reduce(
                        out=hm[:, i].rearrange("p (h d) -> p h d", h=H),
                        in_=mprod[:, i * W4 : (i + 1) * W4].rearrange(
                            "p w (h d) -> p h d w", h=H
                        ),
                        axis=mybir.AxisListType.X,
                        op=mybir.AluOpType.add,
                    )
                hm2 = work.tile([128, SMAX, D], bf16, name=f"hm2{si}", tag="hm2")
                nc.vector.tensor_tensor(
                    out=hm2[:, :n].rearrange("p s (h d) -> p s h d", h=H),
                    in0=hm[:, :n].rearrange("p s (h d) -> p s h d", h=H),
                    in1=rden[:, :n].unsqueeze(3).to_broadcast([128, n, H, HS]),
                    op=mybir.AluOpType.mult,
                )
                # output projection per tile
                for i in range(n):
                    t = t0 + i
                    tp = opsum.tile([128, 128], bf16, space="PSUM", name=f"tp{t}", tag="tp")
                    nc.tensor.transpose(out=tp[:D, :], in_=hm2[:, i], identity=ident[:])
                    hT = work.tile([D, 128], bf16, name=f"hT{t}", tag="hT")
                    nc.any.tensor_copy(out=hT[:], in_=tp[:D, :])
                    o4 = opsum.tile([128, NT * D], f32, space="PSUM", name=f"o4_{t}", tag="o4")
                    nc.tensor.matmul(o4[:], lhsT=hT[:], rhs=wa_s[:], start=True, stop=True)
                    osel = work.tile([128, NT * D], f32, name=f"osel{t}", tag="osel")
                    ohb = (
                        oneh_s[:]
                        .rearrange("p (t f) -> p t f", t=NTILES)[:, t]
                        .unsqueeze(1)
                        .to_broadcast([128, D, NT])
                    )
                    nc.vector.tensor_tensor(
                        out=osel[:].rearrange("p (t d) -> p d t", t=NT),
                        in0=o4[:].rearrange("p (t d) -> p d t", t=NT),
                        in1=ohb,
                        op=mybir.AluOpType.mult,
                    )
                    ot = work.tile([128, D], f32, name=f"ot{t}", tag="ot")
                    nc.vector.tensor_reduce(
                        out=ot[:],
                        in_=osel[:].rearrange("p (t d) -> p d t", t=NT),
                        axis=mybir.AxisListType.X,
                        op=mybir.AluOpType.add,
                    )
                    nc.sync.dma_start(out=outp[t * 128 : (t + 1) * 128, :], in_=ot[:])

            # emission order = scheduler priority: build chunk 0's table first,
            # then interleave later chunks' table slabs with phase 3 of the
            # already-built chunks so DMA/PE/DVE overlap across phases.
            # table build runs TWO chunks ahead of phase 3 so a chunk's last
            # kmtab write lands well before its first gather needs it
            slabs_of_chunk = [
                [si for si, s in enumerate(slabs) if s[0] == h] for h in range(NCH)
            ]
            for em in node_chunk_emitters(0):
                em()
            for em in node_chunk_emitters(1):
                em()
            emit_q()
            for h in range(2, NCH):
                ems = node_chunk_emitters(h)
                tiles = slabs_of_chunk[h - 2]
                ns, ntl = len(ems), len(tiles)
                si_ = ti = 0
                while si_ < ns or ti < ntl:
                    take = (si_ + 1) * ntl <= (ti + 1) * ns
                    if si_ < ns and (take or ti >= ntl):
                        ems[si_]()
                        si_ += 1
                    else:
                        p3_slab(tiles[ti])
                        ti += 1
            for si in slabs_of_chunk[NCH - 2] + slabs_of_chunk[NCH - 1]:
                p3_slab(si)

    nc.compile()
    return nc


def kernel(x, ntype, etype, src, dst, Wk, Wq, Wv, Wa, rel_att, rel_msg, rel_pri):
    import os

    from concourse import bass_utils

    cores, consts = _host_prep(x, ntype, etype, src, dst)
    wkm16, wq416, wa16 = _fold_weights(Wk, Wq, Wv, Wa, rel_att, rel_msg, rel_pri)

    struct_sig = (
        tuple(consts["W"].tolist()),
        consts["NCH"],
        tuple(consts["CHRs"]),
        tuple(consts["R"].ravel().tolist()),
        tuple(consts["slabs"]),
    )
    if "prog" not in _cache or _cache["prog"][0] != struct_sig:
        _cache["prog"] = (struct_sig, _build_program(consts))
    nc = _cache["prog"][1]

    in_maps = [
        dict(
            xpT16=d["xpT16"], wkm16=wkm16, xo4T16=d["xo4T16"], wq416=wq416,
            wa16=wa16, oneh=d["oneh"], kmidx16=d["kmidx16"], abias=d["abias"],
        )
        for d in cores
    ]
    trace_kw = {}
    if os.environ.get("GNN_TRACE") == "1":
        trace_kw = dict(trace=True, tmpdir=os.environ.get("GNN_TRACE_DIR") or None)
    res = bass_utils.run_bass_kernel_spmd(
        nc, in_maps, core_ids=list(range(C)), **trace_kw
    )
    _cache["last_res"] = res

    out = np.zeros((N, D), dtype=np.float32)
    own = consts["own_nodes"]
    for c in range(C):
        oc = res.results[c]["outp"]
        m = own[c] >= 0
        out[own[c][m]] = oc[m]
    out[consts["deg"] == 0] = 0.0
    return out
